# revision 11
# baseline (speedup 1.0000x reference)
"""AuroraAttention (ToMe log-size-bias MHA) on 8 TRN2 NeuronCores.

Sharding: tensor-parallel over heads (2 heads/core) for QKV+attention,
AllToAll re-shard to sequence-parallel (512 tokens/core) for out_proj.
metric (= K averaged over heads) is computed as an independent tiny
projection with host-averaged weights, sequence-sharded (no collective).

Math: softmax(q k^T + log s) @ v  ==  (E @ (s*v)) / (E @ s), E = exp(q k^T),
so the log-size bias never touches the 16M-element logits; it folds into an
augmented V matrix [V0*s | s | V1*s] whose shared middle column yields the
softmax denominators for both heads.
"""

import os
import sys

import numpy as np

sys.path.insert(0, "/opt/trn_rl_repo")

import ml_dtypes  # noqa: E402

from concourse import bacc, mybir, tile  # noqa: E402
from concourse.bass_utils import run_bass_kernel_spmd  # noqa: E402

BF16 = ml_dtypes.bfloat16

L = 4096          # sequence length
D = 1024          # embed dim
H = 16            # heads
HD = 64           # head dim
NC = 8            # cores
JC = D // NC      # 128 q/k/v dims per core (2 heads)
TCH = L // NC     # 512 tokens per core chunk (out/metric rows)
NDT = D // 128    # 8 contraction tiles
NQC = L // 512    # 8 query chunks
NKT = L // 128    # 32 key tiles
SCALE = HD ** -0.5

F32 = mybir.dt.float32
BF = mybir.dt.bfloat16

# cst layout (1, 1472) bf16: bq*SCALE | bk | bv | bm | bo
CST_BQ, CST_BK, CST_BV, CST_BM, CST_BO = 0, 128, 256, 384, 448

_CACHED = {}

# Filled by kernel() after a traced run (test.py reads these).
LAST_EXEC_NS = None
LAST_RESULTS = None


def _install_ntff_shim():
    """Provide antenv.axon_hooks (missing in this image) so
    run_bass_kernel_spmd(trace=True) can NTFF-profile via libaxon_pjrt.so."""
    import contextlib
    import ctypes
    import types

    try:
        from antenv.axon_hooks import get_axon_ntff_profile_hook  # noqa: F401
        return
    except ImportError:
        pass

    so_path = "/opt/axon/libaxon_pjrt.so"
    hook = None
    if os.path.exists(so_path):
        lib = ctypes.CDLL(so_path)
        if hasattr(lib, "axon_start_nrt_profile"):
            lib.axon_start_nrt_profile.argtypes = [
                ctypes.POINTER(ctypes.c_int64),
                ctypes.c_size_t,
            ]
            lib.axon_start_nrt_profile.restype = ctypes.c_int64
            lib.axon_stop_nrt_profile.argtypes = [ctypes.c_char_p]
            lib.axon_stop_nrt_profile.restype = ctypes.c_int64

            @contextlib.contextmanager
            def _hook(output_dir, device_ids):
                import jax

                jax.devices()
                if device_ids:
                    ids = (ctypes.c_int64 * len(device_ids))(*device_ids)
                    rc = lib.axon_start_nrt_profile(ids, len(device_ids))
                else:
                    rc = lib.axon_start_nrt_profile(None, 0)
                if rc != 0:
                    raise RuntimeError(f"axon_start_nrt_profile rc={rc}")
                try:
                    yield
                finally:
                    n = lib.axon_stop_nrt_profile(str(output_dir).encode())
                    print(f"ntff profile: {n} file(s) -> {output_dir}", file=sys.stderr)

            hook = _hook

    import antenv

    mod = types.ModuleType("antenv.axon_hooks")
    _state = {"hook": hook}
    mod.set_axon_ntff_profile_hook = lambda h: _state.__setitem__("hook", h)
    mod.get_axon_ntff_profile_hook = lambda: _state["hook"]
    sys.modules["antenv.axon_hooks"] = mod
    antenv.axon_hooks = mod

    # artifact upload needs cloud creds we don't have; make it a no-op
    import concourse.bass_utils as _bu

    _bu.upload_artifacts = lambda tmpdir: str(tmpdir)


def build():
    nc = bacc.Bacc(None, target_bir_lowering=False)

    hT_d = nc.dram_tensor("hT", [D, L], BF, kind="ExternalInput")
    hTm_d = nc.dram_tensor("hTm", [D, TCH], BF, kind="ExternalInput")
    wqT_d = nc.dram_tensor("wqT", [D, JC], BF, kind="ExternalInput")
    wkT_d = nc.dram_tensor("wkT", [D, JC], BF, kind="ExternalInput")
    wvT_d = nc.dram_tensor("wvT", [D, JC], BF, kind="ExternalInput")
    wmT_d = nc.dram_tensor("wmT", [D, HD], BF, kind="ExternalInput")
    woT_d = nc.dram_tensor("woT", [D, D], BF, kind="ExternalInput")
    cst_d = nc.dram_tensor("cst", [1, 1472], BF, kind="ExternalInput")
    st_d = nc.dram_tensor("st", [128, NKT], F32, kind="ExternalInput")
    id_d = nc.dram_tensor("ident", [128, 128], BF, kind="ExternalInput")

    out_rows = nc.dram_tensor("out_rows", [TCH, D], F32, kind="ExternalOutput")
    metric_rows = nc.dram_tensor("metric_rows", [HD, TCH], F32, kind="ExternalOutput")

    with tile.TileContext(nc) as tc:
        with (
            tc.tile_pool(name="sb", bufs=1) as sb,
            tc.tile_pool(name="work", bufs=2) as work,
            tc.tile_pool(name="ps", bufs=1, space="PSUM") as ps,
            tc.tile_pool(name="dram", bufs=1, space="DRAM") as dpool,
        ):
            # ---------------- constants / weights to SBUF ----------------
            cst_sb = sb.tile([1, 1472], BF, name="cst_sb", tag="cst_sb")
            nc.sync.dma_start(out=cst_sb[:], in_=cst_d[:])
            s_sb = sb.tile([128, NKT], F32, name="s_sb", tag="s_sb")
            nc.sync.dma_start(out=s_sb[:], in_=st_d[:])
            ident_sb = sb.tile([128, 128], BF, name="ident_sb", tag="ident_sb")
            nc.sync.dma_start(out=ident_sb[:], in_=id_d[:])

            ones_sb = sb.tile([1, 512], BF, name="ones_sb", tag="ones_sb")
            nc.vector.memset(ones_sb[:], 1.0)

            wq_sb = sb.tile([128, NDT, JC], BF, name="wq_sb", tag="wq_sb")
            nc.sync.dma_start(out=wq_sb[:], in_=wqT_d[:].rearrange("(a p) j -> p a j", a=NDT))
            wk_sb = sb.tile([128, NDT, JC], BF, name="wk_sb", tag="wk_sb")
            nc.sync.dma_start(out=wk_sb[:], in_=wkT_d[:].rearrange("(a p) j -> p a j", a=NDT))
            wv_sb = sb.tile([128, NDT, JC], BF, name="wv_sb", tag="wv_sb")
            nc.sync.dma_start(out=wv_sb[:], in_=wvT_d[:].rearrange("(a p) j -> p a j", a=NDT))

            hT_sb = []
            for di in range(NDT):
                t = sb.tile([128, L], BF, name=f"hT_sb{di}", tag=f"hT_sb{di}")
                hT_sb.append(t)
            for cg in range(4):
                for di in range(NDT):
                    nc.sync.dma_start(
                        out=hT_sb[di][:, cg * 1024:(cg + 1) * 1024],
                        in_=hT_d[di * 128:(di + 1) * 128, cg * 1024:(cg + 1) * 1024],
                    )

            wm_sb = sb.tile([128, NDT, HD], BF, name="wm_sb", tag="wm_sb")
            nc.sync.dma_start(out=wm_sb[:], in_=wmT_d[:].rearrange("(a p) j -> p a j", a=NDT))
            hTm_sb = sb.tile([128, NDT, TCH], BF, name="hTm_sb", tag="hTm_sb")
            nc.sync.dma_start(out=hTm_sb[:], in_=hTm_d[:].rearrange("(a p) t -> p a t", a=NDT))
            wo_sb = []
            for c in range(NC):
                t = sb.tile([128, D], BF, name=f"wo_sb{c}", tag=f"wo_sb{c}")
                nc.sync.dma_start(out=t[:], in_=woT_d[c * 128:(c + 1) * 128, :])
                wo_sb.append(t)

            # ------- Q^T, K^T, V^T projections (dims on partitions) -------
            QT_sb = sb.tile([128, L], BF, name="QT_sb", tag="QT_sb")
            KT_sb = sb.tile([128, L], BF, name="KT_sb", tag="KT_sb")
            VT_sb = sb.tile([128, L], BF, name="VT_sb", tag="VT_sb")

            def emit_proj_pass(qch, w_sb, boff, dst):
                c0, c1 = 2 * qch, 2 * qch + 1
                pa = ps.tile([128, 512], F32, tag="pj", name=f"pa{qch}_{boff}", bufs=2)
                pb = ps.tile([128, 512], F32, tag="pj", name=f"pb{qch}_{boff}", bufs=2)
                for di in range(NDT):
                    nc.tensor.matmul(
                        pa[:], lhsT=w_sb[:, di, :],
                        rhs=hT_sb[di][:, c0 * 512:(c0 + 1) * 512],
                        start=(di == 0), stop=False,
                    )
                    nc.tensor.matmul(
                        pb[:], lhsT=w_sb[:, di, :],
                        rhs=hT_sb[di][:, c1 * 512:(c1 + 1) * 512],
                        start=(di == 0), stop=False,
                    )
                for p in (pa, pb):
                    nc.tensor.matmul(
                        p[:], lhsT=cst_sb[:, boff:boff + 128],
                        rhs=ones_sb[:, 0:512], start=False, stop=True,
                    )
                nc.vector.tensor_copy(out=dst[:, c0 * 512:(c0 + 1) * 512], in_=pa[:])
                nc.vector.tensor_copy(out=dst[:, c1 * 512:(c1 + 1) * 512], in_=pb[:])

            # ---------------- metric = hTm^T @ wm + bm --------------------
            pm = ps.tile([HD, TCH], F32, tag="pj", name="pm", bufs=2)
            for di in range(NDT):
                nc.tensor.matmul(
                    pm[:], lhsT=wm_sb[:, di, :], rhs=hTm_sb[:, di, :],
                    start=(di == 0), stop=False,
                )
            nc.tensor.matmul(
                pm[:], lhsT=cst_sb[:, CST_BM:CST_BM + HD],
                rhs=ones_sb[:, 0:TCH], start=False, stop=True,
            )
            msb = work.tile([HD, TCH], F32, tag="msb", name="msb")
            nc.vector.tensor_copy(out=msb[:], in_=pm[:])
            nc.sync.dma_start(out=metric_rows[:], in_=msb[:])

            # ------- V natural via TensorE transpose, + aug columns -------
            # Vaug[:, tt, :] = [V0*s (0:64) | s (64) | V1*s (65:129) | s (129)]
            Vaug_sb = sb.tile([128, NKT, 130], BF, name="Vaug_sb", tag="Vaug_sb")

            def emit_vaug(tt):
                pv = ps.tile([128, JC], BF, tag="pj", name=f"pv{tt}", bufs=2)
                nc.tensor.transpose(
                    pv[:], VT_sb[:, tt * 128:(tt + 1) * 128], ident_sb[:]
                )
                sc = s_sb[:, tt:tt + 1]
                nc.vector.tensor_scalar(
                    Vaug_sb[:, tt, 0:64], pv[:, 0:64], sc, None, mybir.AluOpType.mult
                )
                nc.vector.tensor_scalar(
                    Vaug_sb[:, tt, 65:129], pv[:, 64:128], sc, None, mybir.AluOpType.mult
                )
                nc.vector.tensor_copy(out=Vaug_sb[:, tt, 64:65], in_=sc)
                nc.vector.tensor_copy(out=Vaug_sb[:, tt, 129:130], in_=sc)

            # ---------------- attention ----------------------------------
            attnT_sb = sb.tile([128, L], BF, name="attnT_sb", tag="attnT_sb")

            def emit_kt(qc, kt, u0, u1):
                stp = ps.tile([128, 1024], F32, tag="st", name=f"stp{qc}_{kt}", bufs=2)
                nc.tensor.matmul(
                    stp[:, 0:512],
                    lhsT=KT_sb[0:64, kt * 128:(kt + 1) * 128],
                    rhs=QT_sb[0:64, qc * 512:(qc + 1) * 512],
                    start=True, stop=True,
                )
                nc.tensor.matmul(
                    stp[:, 512:1024],
                    lhsT=KT_sb[64:128, kt * 128:(kt + 1) * 128],
                    rhs=QT_sb[64:128, qc * 512:(qc + 1) * 512],
                    start=True, stop=True,
                )
                sts = work.tile([128, 1024], BF, tag="sts", name=f"sts{qc}_{kt}", bufs=3)
                nc.scalar.activation(
                    out=sts[:], in_=stp[:], func=mybir.ActivationFunctionType.Exp
                )
                nc.tensor.matmul(
                    u0[:], lhsT=Vaug_sb[:, kt, 0:65], rhs=sts[:, 0:512],
                    start=(kt == 0), stop=(kt == NKT - 1),
                )
                nc.tensor.matmul(
                    u1[:], lhsT=Vaug_sb[:, kt, 65:130], rhs=sts[:, 512:1024],
                    start=(kt == 0), stop=(kt == NKT - 1),
                )

            def emit_u_copy(qc, u0, u1):
                # copy U out of PSUM right away so the u slots recycle fast
                stash = []
                for h, u in enumerate((u0, u1)):
                    usb = work.tile(
                        [65, 512], F32, tag="usb", name=f"usb{qc}_{h}", bufs=4
                    )
                    nc.vector.tensor_copy(out=usb[:], in_=u[:])
                    stash.append(usb)
                return stash

            def emit_norm(qc, stash):
                # attnT = U / denom; denom row 64; PE not involved
                for h, usb in enumerate(stash):
                    rec = work.tile([1, 512], F32, tag="rec", name=f"rec{qc}_{h}")
                    nc.vector.reciprocal(out=rec[:], in_=usb[64:65, :])
                    bcs = work.tile([64, 512], F32, tag="bcs", name=f"bcs{qc}_{h}")
                    nc.gpsimd.partition_broadcast(bcs[:], rec[:])
                    nc.vector.tensor_tensor(
                        attnT_sb[64 * h:64 * h + 64, qc * 512:(qc + 1) * 512],
                        usb[0:64, :], bcs[:], mybir.AluOpType.mult,
                    )

            # ---- pipelined A2A + out_proj: 4 groups of 2 q-chunks ----
            # group g covers token cols [1024g, 1024g+1024); rank r gets the
            # 128-col slice at 1024g + 128r. core r's out_rows row-block g
            # holds global tokens [1024g + 128r, 1024g + 128r + 128).
            a2a_ins, a2a_outs, L_gs = [], [], []
            for g in range(4):
                t_in = dpool.tile([NC, 128, 128], BF, name=f"a2a_in{g}", tag=f"a2a_in{g}")
                t_out = dpool.tile([NC, 128, 128], BF, name=f"a2a_out{g}", tag=f"a2a_out{g}")
                a2a_ins.append(t_in)
                a2a_outs.append(t_out)

            def emit_a2a(g):
                base = 1024 * g
                nc.sync.dma_start(
                    out=a2a_ins[g][:].rearrange("r p q -> p r q"),
                    in_=attnT_sb[:, base:base + 1024].rearrange(
                        "p (r q) -> p r q", r=NC
                    ),
                )
                nc.gpsimd.collective_compute(
                    "AllToAll",
                    mybir.AluOpType.bypass,
                    replica_groups=[list(range(NC))],
                    ins=[a2a_ins[g].opt()],
                    outs=[a2a_outs[g].opt()],
                )
                L_g = sb.tile([128, NC, 128], BF, name=f"L_sb{g}", tag=f"L_sb{g}")
                nc.sync.dma_start(
                    out=L_g[:], in_=a2a_outs[g][:].rearrange("r p q -> p r q")
                )
                L_gs.append(L_g)

            def emit_outproj(g):
                L_g = L_gs[g]
                for ec in range(D // 512):
                    po = ps.tile([128, 512], F32, tag="pj", name=f"po{g}_{ec}", bufs=2)
                    for c in range(NC):
                        nc.tensor.matmul(
                            po[:], lhsT=L_g[:, c, :],
                            rhs=wo_sb[c][:, ec * 512:(ec + 1) * 512],
                            start=(c == 0), stop=False,
                        )
                    nc.tensor.matmul(
                        po[:], lhsT=ones_sb[:, 0:128],
                        rhs=cst_sb[:, CST_BO + ec * 512:CST_BO + (ec + 1) * 512],
                        start=False, stop=True,
                    )
                    osb = work.tile([128, 512], F32, tag="osb", name=f"osb{g}_{ec}")
                    nc.vector.tensor_copy(out=osb[:], in_=po[:])
                    nc.sync.dma_start(
                        out=out_rows[g * 128:(g + 1) * 128, ec * 512:(ec + 1) * 512],
                        in_=osb[:],
                    )

            # proj passes interleaved with attention qc=0 so ScalarE's
            # exp stream (the critical path) starts as early as possible
            u0_0 = ps.tile([65, 512], F32, tag="u", name="u0_0", bufs=2)
            u1_0 = ps.tile([65, 512], F32, tag="u", name="u1_0", bufs=2)
            for qch in range(NQC // 2):
                for w_sb, boff, dst in (
                    (wq_sb, CST_BQ, QT_sb),
                    (wk_sb, CST_BK, KT_sb),
                    (wv_sb, CST_BV, VT_sb),
                ):
                    if qch > 0 and w_sb is wq_sb:
                        continue
                    emit_proj_pass(qch, w_sb, boff, dst)
                for tt in range(8 * qch, 8 * qch + 8):
                    emit_vaug(tt)
                for kt in range(8 * qch, 8 * qch + 8):
                    emit_kt(0, kt, u0_0, u1_0)
            for qch in range(1, NQC // 2):
                emit_proj_pass(qch, wq_sb, CST_BQ, QT_sb)
            prev = emit_u_copy(0, u0_0, u1_0)
            for qc in range(1, NQC):
                u0 = ps.tile([65, 512], F32, tag="u", name=f"u0_{qc}", bufs=2)
                u1 = ps.tile([65, 512], F32, tag="u", name=f"u1_{qc}", bufs=2)
                for kt in range(NKT):
                    emit_kt(qc, kt, u0, u1)
                    if kt == 6 and prev is not None:
                        emit_norm(qc - 1, prev)
                        prev = None
                        if qc % 2 == 0 and qc >= 2:
                            emit_a2a((qc - 1) // 2)
                    if kt == 20 and qc % 2 == 1 and qc >= 3:
                        emit_outproj((qc - 3) // 2)
                prev = emit_u_copy(qc, u0, u1)
            emit_norm(NQC - 1, prev)
            emit_a2a(3)
            emit_outproj(3)

    if not nc.is_finalized():
        nc.finalize()
    return nc


def _prep_inputs(hidden_states, size, wq, bq, wk, bk, wv, bv, wo, bo):
    h = np.asarray(hidden_states, dtype=np.float32)[0]          # (L, D)
    s = np.asarray(size, dtype=np.float32)[0, 0]                # (L,)
    wq = np.asarray(wq, dtype=np.float32)
    wk = np.asarray(wk, dtype=np.float32)
    wv = np.asarray(wv, dtype=np.float32)
    wo = np.asarray(wo, dtype=np.float32)
    bq = np.asarray(bq, dtype=np.float32)
    bk = np.asarray(bk, dtype=np.float32)
    bv = np.asarray(bv, dtype=np.float32)
    bo = np.asarray(bo, dtype=np.float32)

    hT = np.ascontiguousarray(h.T).astype(BF16)                 # (D, L)
    woT = np.ascontiguousarray(wo.T).astype(BF16)               # (D, D)
    wm = wk.reshape(H, HD, D).mean(axis=0)                      # (HD, D)
    bm = bk.reshape(H, HD).mean(axis=0)                         # (HD,)
    wmT = np.ascontiguousarray(wm.T).astype(BF16)               # (D, HD)
    s_t = np.ascontiguousarray(s.reshape(NKT, 128).T).astype(np.float32)
    ident = np.eye(128, dtype=np.float32).astype(BF16)

    in_maps = []
    for c in range(NC):
        rows = slice(c * JC, (c + 1) * JC)
        wqc = wq[rows] * SCALE
        cst = np.concatenate(
            [bq[rows] * SCALE, bk[rows], bv[rows], bm, bo]
        ).reshape(1, -1).astype(BF16)
        in_maps.append({
            "hT": hT,
            "hTm": np.ascontiguousarray(hT[:, c * TCH:(c + 1) * TCH]),
            "wqT": np.ascontiguousarray(wqc.T).astype(BF16),
            "wkT": np.ascontiguousarray(wk[rows].T).astype(BF16),
            "wvT": np.ascontiguousarray(wv[rows].T).astype(BF16),
            "wmT": wmT,
            "woT": woT,
            "cst": cst,
            "st": s_t,
            "ident": ident,
        })
    return in_maps


def kernel(hidden_states, size, wq, bq, wk, bk, wv, bv, wo, bo):
    global LAST_EXEC_NS, LAST_RESULTS
    if "nc" not in _CACHED:
        _CACHED["nc"] = build()
    nc = _CACHED["nc"]

    in_maps = _prep_inputs(hidden_states, size, wq, bq, wk, bk, wv, bv, wo, bo)

    trace = bool(int(os.environ.get("AURORA_TRACE", "0")))
    if trace:
        _install_ntff_shim()
    res = run_bass_kernel_spmd(
        nc, in_maps, core_ids=list(range(NC)), trace=trace
    )
    LAST_RESULTS = res
    LAST_EXEC_NS = res.exec_time_ns

    outs = res.results
    out = np.empty((1, L, D), dtype=np.float32)
    for r in range(NC):
        rows = np.asarray(outs[r]["out_rows"], dtype=np.float32)
        for g in range(4):
            base = 1024 * g + 128 * r
            out[0, base:base + 128, :] = rows[g * 128:(g + 1) * 128, :]
    met = np.concatenate(
        [np.asarray(outs[c]["metric_rows"], dtype=np.float32) for c in range(NC)],
        axis=1,
    ).T[None]                                                   # (1, L, HD)
    return out, met


# revision 12
# speedup vs baseline: 1.0260x; 1.0260x over previous
"""AuroraAttention (ToMe log-size-bias MHA) on 8 TRN2 NeuronCores.

Sharding: tensor-parallel over heads (2 heads/core) for QKV+attention,
AllToAll re-shard to sequence-parallel (512 tokens/core) for out_proj.
metric (= K averaged over heads) is computed as an independent tiny
projection with host-averaged weights, sequence-sharded (no collective).

Math: softmax(q k^T + log s) @ v  ==  (E @ (s*v)) / (E @ s), E = exp(q k^T),
so the log-size bias never touches the 16M-element logits; it folds into an
augmented V matrix [V0*s | s | V1*s] whose shared middle column yields the
softmax denominators for both heads.
"""

import os
import sys

import numpy as np

sys.path.insert(0, "/opt/trn_rl_repo")

import ml_dtypes  # noqa: E402

from concourse import bacc, mybir, tile  # noqa: E402
from concourse.bass_utils import run_bass_kernel_spmd  # noqa: E402

BF16 = ml_dtypes.bfloat16

L = 4096          # sequence length
D = 1024          # embed dim
H = 16            # heads
HD = 64           # head dim
NC = 8            # cores
JC = D // NC      # 128 q/k/v dims per core (2 heads)
TCH = L // NC     # 512 tokens per core chunk (out/metric rows)
NDT = D // 128    # 8 contraction tiles
NQC = L // 512    # 8 query chunks
NKT = L // 128    # 32 key tiles
SCALE = HD ** -0.5

F32 = mybir.dt.float32
BF = mybir.dt.bfloat16

# cst layout (1, 1472) bf16: bq*SCALE | bk | bv | bm | bo
CST_BQ, CST_BK, CST_BV, CST_BM, CST_BO = 0, 128, 256, 384, 448

_CACHED = {}

# Filled by kernel() after a traced run (test.py reads these).
LAST_EXEC_NS = None
LAST_RESULTS = None


def _install_ntff_shim():
    """Provide antenv.axon_hooks (missing in this image) so
    run_bass_kernel_spmd(trace=True) can NTFF-profile via libaxon_pjrt.so."""
    import contextlib
    import ctypes
    import types

    try:
        from antenv.axon_hooks import get_axon_ntff_profile_hook  # noqa: F401
        return
    except ImportError:
        pass

    so_path = "/opt/axon/libaxon_pjrt.so"
    hook = None
    if os.path.exists(so_path):
        lib = ctypes.CDLL(so_path)
        if hasattr(lib, "axon_start_nrt_profile"):
            lib.axon_start_nrt_profile.argtypes = [
                ctypes.POINTER(ctypes.c_int64),
                ctypes.c_size_t,
            ]
            lib.axon_start_nrt_profile.restype = ctypes.c_int64
            lib.axon_stop_nrt_profile.argtypes = [ctypes.c_char_p]
            lib.axon_stop_nrt_profile.restype = ctypes.c_int64

            @contextlib.contextmanager
            def _hook(output_dir, device_ids):
                import jax

                jax.devices()
                if device_ids:
                    ids = (ctypes.c_int64 * len(device_ids))(*device_ids)
                    rc = lib.axon_start_nrt_profile(ids, len(device_ids))
                else:
                    rc = lib.axon_start_nrt_profile(None, 0)
                if rc != 0:
                    raise RuntimeError(f"axon_start_nrt_profile rc={rc}")
                try:
                    yield
                finally:
                    n = lib.axon_stop_nrt_profile(str(output_dir).encode())
                    print(f"ntff profile: {n} file(s) -> {output_dir}", file=sys.stderr)

            hook = _hook

    import antenv

    mod = types.ModuleType("antenv.axon_hooks")
    _state = {"hook": hook}
    mod.set_axon_ntff_profile_hook = lambda h: _state.__setitem__("hook", h)
    mod.get_axon_ntff_profile_hook = lambda: _state["hook"]
    sys.modules["antenv.axon_hooks"] = mod
    antenv.axon_hooks = mod

    # artifact upload needs cloud creds we don't have; make it a no-op
    import concourse.bass_utils as _bu

    _bu.upload_artifacts = lambda tmpdir: str(tmpdir)


def build():
    nc = bacc.Bacc(None, target_bir_lowering=False)

    hT_d = nc.dram_tensor("hT", [D, L], BF, kind="ExternalInput")
    hTm_d = nc.dram_tensor("hTm", [D, TCH], BF, kind="ExternalInput")
    wqT_d = nc.dram_tensor("wqT", [D, JC], BF, kind="ExternalInput")
    wkT_d = nc.dram_tensor("wkT", [D, JC], BF, kind="ExternalInput")
    wvT_d = nc.dram_tensor("wvT", [D, JC], BF, kind="ExternalInput")
    wmT_d = nc.dram_tensor("wmT", [D, HD], BF, kind="ExternalInput")
    woT_d = nc.dram_tensor("woT", [D, D], BF, kind="ExternalInput")
    cst_d = nc.dram_tensor("cst", [1, 1472], BF, kind="ExternalInput")
    st_d = nc.dram_tensor("st", [128, NKT], F32, kind="ExternalInput")
    id_d = nc.dram_tensor("ident", [128, 128], BF, kind="ExternalInput")

    out_rows = nc.dram_tensor("out_rows", [TCH, D], F32, kind="ExternalOutput")
    metric_rows = nc.dram_tensor("metric_rows", [HD, TCH], F32, kind="ExternalOutput")

    with tile.TileContext(nc) as tc:
        with (
            tc.tile_pool(name="sb", bufs=1) as sb,
            tc.tile_pool(name="work", bufs=2) as work,
            tc.tile_pool(name="ps", bufs=1, space="PSUM") as ps,
            tc.tile_pool(name="dram", bufs=1, space="DRAM") as dpool,
        ):
            # ---------------- constants / weights to SBUF ----------------
            cst_sb = sb.tile([1, 1472], BF, name="cst_sb", tag="cst_sb")
            nc.sync.dma_start(out=cst_sb[:], in_=cst_d[:])
            s_sb = sb.tile([128, NKT], F32, name="s_sb", tag="s_sb")
            nc.sync.dma_start(out=s_sb[:], in_=st_d[:])
            ident_sb = sb.tile([128, 128], BF, name="ident_sb", tag="ident_sb")
            nc.sync.dma_start(out=ident_sb[:], in_=id_d[:])

            ones_sb = sb.tile([1, 512], BF, name="ones_sb", tag="ones_sb")
            nc.vector.memset(ones_sb[:], 1.0)

            wq_sb = sb.tile([128, NDT, JC], BF, name="wq_sb", tag="wq_sb")
            nc.sync.dma_start(out=wq_sb[:], in_=wqT_d[:].rearrange("(a p) j -> p a j", a=NDT))
            wk_sb = sb.tile([128, NDT, JC], BF, name="wk_sb", tag="wk_sb")
            nc.sync.dma_start(out=wk_sb[:], in_=wkT_d[:].rearrange("(a p) j -> p a j", a=NDT))
            wv_sb = sb.tile([128, NDT, JC], BF, name="wv_sb", tag="wv_sb")
            nc.sync.dma_start(out=wv_sb[:], in_=wvT_d[:].rearrange("(a p) j -> p a j", a=NDT))

            hT_sb = []
            for di in range(NDT):
                t = sb.tile([128, L], BF, name=f"hT_sb{di}", tag=f"hT_sb{di}")
                nc.sync.dma_start(out=t[:], in_=hT_d[di * 128:(di + 1) * 128, :])
                hT_sb.append(t)

            wm_sb = sb.tile([128, NDT, HD], BF, name="wm_sb", tag="wm_sb")
            nc.sync.dma_start(out=wm_sb[:], in_=wmT_d[:].rearrange("(a p) j -> p a j", a=NDT))
            hTm_sb = sb.tile([128, NDT, TCH], BF, name="hTm_sb", tag="hTm_sb")
            nc.sync.dma_start(out=hTm_sb[:], in_=hTm_d[:].rearrange("(a p) t -> p a t", a=NDT))
            wo_sb = []
            for c in range(NC):
                t = sb.tile([128, D], BF, name=f"wo_sb{c}", tag=f"wo_sb{c}")
                nc.sync.dma_start(out=t[:], in_=woT_d[c * 128:(c + 1) * 128, :])
                wo_sb.append(t)

            # ------- Q^T, K^T, V^T projections (dims on partitions) -------
            QT_sb = sb.tile([128, L], BF, name="QT_sb", tag="QT_sb")
            KT_sb = sb.tile([128, L], BF, name="KT_sb", tag="KT_sb")
            VT_sb = sb.tile([128, L], BF, name="VT_sb", tag="VT_sb")

            def emit_proj_pass(qch, w_sb, boff, dst):
                cs = [4 * qch + j for j in range(4)]
                pt = []
                for j, c in enumerate(cs):
                    tag = "pj" if j < 2 else "u"
                    pt.append(ps.tile(
                        [128, 512], F32, tag=tag, name=f"p{qch}_{boff}_{j}", bufs=2
                    ))
                for di in range(NDT):
                    for j in range(4):
                        nc.tensor.matmul(
                            pt[j][:], lhsT=w_sb[:, di, :],
                            rhs=hT_sb[di][:, cs[j] * 512:(cs[j] + 1) * 512],
                            start=(di == 0), stop=False,
                        )
                for j in range(4):
                    nc.tensor.matmul(
                        pt[j][:], lhsT=cst_sb[:, boff:boff + 128],
                        rhs=ones_sb[:, 0:512], start=False, stop=True,
                    )
                for j in range(4):
                    nc.vector.tensor_copy(
                        out=dst[:, cs[j] * 512:(cs[j] + 1) * 512], in_=pt[j][:]
                    )

            # ---------------- metric = hTm^T @ wm + bm --------------------
            pm = ps.tile([HD, TCH], F32, tag="pj", name="pm", bufs=2)
            for di in range(NDT):
                nc.tensor.matmul(
                    pm[:], lhsT=wm_sb[:, di, :], rhs=hTm_sb[:, di, :],
                    start=(di == 0), stop=False,
                )
            nc.tensor.matmul(
                pm[:], lhsT=cst_sb[:, CST_BM:CST_BM + HD],
                rhs=ones_sb[:, 0:TCH], start=False, stop=True,
            )
            msb = work.tile([HD, TCH], F32, tag="msb", name="msb")
            nc.vector.tensor_copy(out=msb[:], in_=pm[:])
            nc.sync.dma_start(out=metric_rows[:], in_=msb[:])

            # ------- V natural via TensorE transpose, + aug columns -------
            # Vaug[:, tt, :] = [V0*s (0:64) | s (64) | V1*s (65:129) | s (129)]
            Vaug_sb = sb.tile([128, NKT, 130], BF, name="Vaug_sb", tag="Vaug_sb")

            def emit_vaug(tt):
                pv = ps.tile([128, JC], BF, tag="pj", name=f"pv{tt}", bufs=2)
                nc.tensor.transpose(
                    pv[:], VT_sb[:, tt * 128:(tt + 1) * 128], ident_sb[:]
                )
                sc = s_sb[:, tt:tt + 1]
                nc.vector.tensor_scalar(
                    Vaug_sb[:, tt, 0:64], pv[:, 0:64], sc, None, mybir.AluOpType.mult
                )
                nc.vector.tensor_scalar(
                    Vaug_sb[:, tt, 65:129], pv[:, 64:128], sc, None, mybir.AluOpType.mult
                )
                nc.vector.tensor_copy(out=Vaug_sb[:, tt, 64:65], in_=sc)
                nc.vector.tensor_copy(out=Vaug_sb[:, tt, 129:130], in_=sc)

            # ---------------- attention ----------------------------------
            attnT_sb = sb.tile([128, L], BF, name="attnT_sb", tag="attnT_sb")

            def emit_kt(qc, kt, u0, u1):
                stp = ps.tile([128, 1024], F32, tag="st", name=f"stp{qc}_{kt}", bufs=2)
                nc.tensor.matmul(
                    stp[:, 0:512],
                    lhsT=KT_sb[0:64, kt * 128:(kt + 1) * 128],
                    rhs=QT_sb[0:64, qc * 512:(qc + 1) * 512],
                    start=True, stop=True,
                )
                nc.tensor.matmul(
                    stp[:, 512:1024],
                    lhsT=KT_sb[64:128, kt * 128:(kt + 1) * 128],
                    rhs=QT_sb[64:128, qc * 512:(qc + 1) * 512],
                    start=True, stop=True,
                )
                sts = work.tile([128, 1024], BF, tag="sts", name=f"sts{qc}_{kt}", bufs=3)
                nc.scalar.activation(
                    out=sts[:], in_=stp[:], func=mybir.ActivationFunctionType.Exp
                )
                nc.tensor.matmul(
                    u0[:], lhsT=Vaug_sb[:, kt, 0:65], rhs=sts[:, 0:512],
                    start=(kt == 0), stop=(kt == NKT - 1),
                )
                nc.tensor.matmul(
                    u1[:], lhsT=Vaug_sb[:, kt, 65:130], rhs=sts[:, 512:1024],
                    start=(kt == 0), stop=(kt == NKT - 1),
                )

            def emit_u_copy(qc, u0, u1):
                # copy U out of PSUM right away so the u slots recycle fast
                stash = []
                for h, u in enumerate((u0, u1)):
                    usb = work.tile(
                        [65, 512], F32, tag="usb", name=f"usb{qc}_{h}", bufs=4
                    )
                    nc.vector.tensor_copy(out=usb[:], in_=u[:])
                    stash.append(usb)
                return stash

            def emit_norm(qc, stash):
                # attnT = U / denom; denom row 64; PE not involved
                for h, usb in enumerate(stash):
                    rec = work.tile([1, 512], F32, tag="rec", name=f"rec{qc}_{h}")
                    nc.vector.reciprocal(out=rec[:], in_=usb[64:65, :])
                    bcs = work.tile([64, 512], F32, tag="bcs", name=f"bcs{qc}_{h}")
                    nc.gpsimd.partition_broadcast(bcs[:], rec[:])
                    nc.vector.tensor_tensor(
                        attnT_sb[64 * h:64 * h + 64, qc * 512:(qc + 1) * 512],
                        usb[0:64, :], bcs[:], mybir.AluOpType.mult,
                    )

            # ---- pipelined A2A + out_proj: 4 groups of 2 q-chunks ----
            # group g covers token cols [1024g, 1024g+1024); rank r gets the
            # 128-col slice at 1024g + 128r. core r's out_rows row-block g
            # holds global tokens [1024g + 128r, 1024g + 128r + 128).
            a2a_ins, a2a_outs, L_gs = [], [], []
            for g in range(4):
                t_in = dpool.tile([NC, 128, 128], BF, name=f"a2a_in{g}", tag=f"a2a_in{g}")
                t_out = dpool.tile([NC, 128, 128], BF, name=f"a2a_out{g}", tag=f"a2a_out{g}")
                a2a_ins.append(t_in)
                a2a_outs.append(t_out)

            def emit_a2a(g):
                base = 1024 * g
                nc.sync.dma_start(
                    out=a2a_ins[g][:].rearrange("r p q -> p r q"),
                    in_=attnT_sb[:, base:base + 1024].rearrange(
                        "p (r q) -> p r q", r=NC
                    ),
                )
                nc.gpsimd.collective_compute(
                    "AllToAll",
                    mybir.AluOpType.bypass,
                    replica_groups=[list(range(NC))],
                    ins=[a2a_ins[g].opt()],
                    outs=[a2a_outs[g].opt()],
                )
                L_g = sb.tile([128, NC, 128], BF, name=f"L_sb{g}", tag=f"L_sb{g}")
                nc.sync.dma_start(
                    out=L_g[:], in_=a2a_outs[g][:].rearrange("r p q -> p r q")
                )
                L_gs.append(L_g)

            def emit_outproj(g):
                L_g = L_gs[g]
                po = [
                    ps.tile([128, 512], F32, tag="pj", name=f"po{g}_{ec}", bufs=2)
                    for ec in range(D // 512)
                ]
                for c in range(NC):
                    for ec in range(D // 512):
                        nc.tensor.matmul(
                            po[ec][:], lhsT=L_g[:, c, :],
                            rhs=wo_sb[c][:, ec * 512:(ec + 1) * 512],
                            start=(c == 0), stop=False,
                        )
                for ec in range(D // 512):
                    nc.tensor.matmul(
                        po[ec][:], lhsT=ones_sb[:, 0:128],
                        rhs=cst_sb[:, CST_BO + ec * 512:CST_BO + (ec + 1) * 512],
                        start=False, stop=True,
                    )
                for ec in range(D // 512):
                    osb = work.tile([128, 512], F32, tag="osb", name=f"osb{g}_{ec}")
                    nc.vector.tensor_copy(out=osb[:], in_=po[ec][:])
                    nc.sync.dma_start(
                        out=out_rows[g * 128:(g + 1) * 128, ec * 512:(ec + 1) * 512],
                        in_=osb[:],
                    )

            for qch in range(NQC // 4):
                for w_sb, boff, dst in (
                    (wq_sb, CST_BQ, QT_sb),
                    (wk_sb, CST_BK, KT_sb),
                    (wv_sb, CST_BV, VT_sb),
                ):
                    emit_proj_pass(qch, w_sb, boff, dst)
            for tt in range(NKT):
                emit_vaug(tt)
            prev = None
            for qc in range(NQC):
                u0 = ps.tile([65, 512], F32, tag="u", name=f"u0_{qc}", bufs=2)
                u1 = ps.tile([65, 512], F32, tag="u", name=f"u1_{qc}", bufs=2)
                for kt in range(NKT):
                    emit_kt(qc, kt, u0, u1)
                    if kt == 6 and prev is not None:
                        emit_norm(qc - 1, prev)
                        prev = None
                        if qc % 2 == 0 and qc >= 2:
                            emit_a2a((qc - 1) // 2)
                    if kt == 20 and qc % 2 == 1 and qc >= 3:
                        emit_outproj((qc - 3) // 2)
                prev = emit_u_copy(qc, u0, u1)
            emit_norm(NQC - 1, prev)
            emit_a2a(3)
            emit_outproj(3)

    if not nc.is_finalized():
        nc.finalize()
    return nc


def _prep_inputs(hidden_states, size, wq, bq, wk, bk, wv, bv, wo, bo):
    h = np.asarray(hidden_states, dtype=np.float32)[0]          # (L, D)
    s = np.asarray(size, dtype=np.float32)[0, 0]                # (L,)
    wq = np.asarray(wq, dtype=np.float32)
    wk = np.asarray(wk, dtype=np.float32)
    wv = np.asarray(wv, dtype=np.float32)
    wo = np.asarray(wo, dtype=np.float32)
    bq = np.asarray(bq, dtype=np.float32)
    bk = np.asarray(bk, dtype=np.float32)
    bv = np.asarray(bv, dtype=np.float32)
    bo = np.asarray(bo, dtype=np.float32)

    hT = np.ascontiguousarray(h.T).astype(BF16)                 # (D, L)
    woT = np.ascontiguousarray(wo.T).astype(BF16)               # (D, D)
    wm = wk.reshape(H, HD, D).mean(axis=0)                      # (HD, D)
    bm = bk.reshape(H, HD).mean(axis=0)                         # (HD,)
    wmT = np.ascontiguousarray(wm.T).astype(BF16)               # (D, HD)
    s_t = np.ascontiguousarray(s.reshape(NKT, 128).T).astype(np.float32)
    ident = np.eye(128, dtype=np.float32).astype(BF16)

    in_maps = []
    for c in range(NC):
        rows = slice(c * JC, (c + 1) * JC)
        wqc = wq[rows] * SCALE
        cst = np.concatenate(
            [bq[rows] * SCALE, bk[rows], bv[rows], bm, bo]
        ).reshape(1, -1).astype(BF16)
        in_maps.append({
            "hT": hT,
            "hTm": np.ascontiguousarray(hT[:, c * TCH:(c + 1) * TCH]),
            "wqT": np.ascontiguousarray(wqc.T).astype(BF16),
            "wkT": np.ascontiguousarray(wk[rows].T).astype(BF16),
            "wvT": np.ascontiguousarray(wv[rows].T).astype(BF16),
            "wmT": wmT,
            "woT": woT,
            "cst": cst,
            "st": s_t,
            "ident": ident,
        })
    return in_maps


def kernel(hidden_states, size, wq, bq, wk, bk, wv, bv, wo, bo):
    global LAST_EXEC_NS, LAST_RESULTS
    if "nc" not in _CACHED:
        _CACHED["nc"] = build()
    nc = _CACHED["nc"]

    in_maps = _prep_inputs(hidden_states, size, wq, bq, wk, bk, wv, bv, wo, bo)

    trace = bool(int(os.environ.get("AURORA_TRACE", "0")))
    if trace:
        _install_ntff_shim()
    res = run_bass_kernel_spmd(
        nc, in_maps, core_ids=list(range(NC)), trace=trace
    )
    LAST_RESULTS = res
    LAST_EXEC_NS = res.exec_time_ns

    outs = res.results
    out = np.empty((1, L, D), dtype=np.float32)
    for r in range(NC):
        rows = np.asarray(outs[r]["out_rows"], dtype=np.float32)
        for g in range(4):
            base = 1024 * g + 128 * r
            out[0, base:base + 128, :] = rows[g * 128:(g + 1) * 128, :]
    met = np.concatenate(
        [np.asarray(outs[c]["metric_rows"], dtype=np.float32) for c in range(NC)],
        axis=1,
    ).T[None]                                                   # (1, L, HD)
    return out, met


# revision 13
# speedup vs baseline: 1.0751x; 1.0479x over previous
"""AuroraAttention (ToMe log-size-bias MHA) on 8 TRN2 NeuronCores.

Sharding: tensor-parallel over heads (2 heads/core) for QKV+attention,
AllToAll re-shard to sequence-parallel (512 tokens/core) for out_proj.
metric (= K averaged over heads) is computed as an independent tiny
projection with host-averaged weights, sequence-sharded (no collective).

Math: softmax(q k^T + log s) @ v  ==  (E @ (s*v)) / (E @ s), E = exp(q k^T),
so the log-size bias never touches the 16M-element logits; it folds into an
augmented V matrix [V0*s | s | V1*s] whose shared middle column yields the
softmax denominators for both heads.
"""

import os
import sys

import numpy as np

sys.path.insert(0, "/opt/trn_rl_repo")

import ml_dtypes  # noqa: E402

from concourse import bacc, mybir, tile  # noqa: E402
from concourse.bass_utils import run_bass_kernel_spmd  # noqa: E402

BF16 = ml_dtypes.bfloat16

L = 4096          # sequence length
D = 1024          # embed dim
H = 16            # heads
HD = 64           # head dim
NC = 8            # cores
JC = D // NC      # 128 q/k/v dims per core (2 heads)
TCH = L // NC     # 512 tokens per core chunk (out/metric rows)
NDT = D // 128    # 8 contraction tiles
NQC = L // 512    # 8 query chunks
NKT = L // 128    # 32 key tiles
SCALE = HD ** -0.5

F32 = mybir.dt.float32
BF = mybir.dt.bfloat16

# cst layout (1, 1472) bf16: bq*SCALE | bk | bv | bm | bo
CST_BQ, CST_BK, CST_BV, CST_BM, CST_BO = 0, 128, 256, 384, 448

_CACHED = {}

# Filled by kernel() after a traced run (test.py reads these).
LAST_EXEC_NS = None
LAST_RESULTS = None


def _install_ntff_shim():
    """Provide antenv.axon_hooks (missing in this image) so
    run_bass_kernel_spmd(trace=True) can NTFF-profile via libaxon_pjrt.so."""
    import contextlib
    import ctypes
    import types

    try:
        from antenv.axon_hooks import get_axon_ntff_profile_hook  # noqa: F401
        return
    except ImportError:
        pass

    so_path = "/opt/axon/libaxon_pjrt.so"
    hook = None
    if os.path.exists(so_path):
        lib = ctypes.CDLL(so_path)
        if hasattr(lib, "axon_start_nrt_profile"):
            lib.axon_start_nrt_profile.argtypes = [
                ctypes.POINTER(ctypes.c_int64),
                ctypes.c_size_t,
            ]
            lib.axon_start_nrt_profile.restype = ctypes.c_int64
            lib.axon_stop_nrt_profile.argtypes = [ctypes.c_char_p]
            lib.axon_stop_nrt_profile.restype = ctypes.c_int64

            @contextlib.contextmanager
            def _hook(output_dir, device_ids):
                import jax

                jax.devices()
                if device_ids:
                    ids = (ctypes.c_int64 * len(device_ids))(*device_ids)
                    rc = lib.axon_start_nrt_profile(ids, len(device_ids))
                else:
                    rc = lib.axon_start_nrt_profile(None, 0)
                if rc != 0:
                    raise RuntimeError(f"axon_start_nrt_profile rc={rc}")
                try:
                    yield
                finally:
                    n = lib.axon_stop_nrt_profile(str(output_dir).encode())
                    print(f"ntff profile: {n} file(s) -> {output_dir}", file=sys.stderr)

            hook = _hook

    import antenv

    mod = types.ModuleType("antenv.axon_hooks")
    _state = {"hook": hook}
    mod.set_axon_ntff_profile_hook = lambda h: _state.__setitem__("hook", h)
    mod.get_axon_ntff_profile_hook = lambda: _state["hook"]
    sys.modules["antenv.axon_hooks"] = mod
    antenv.axon_hooks = mod

    # artifact upload needs cloud creds we don't have; make it a no-op
    import concourse.bass_utils as _bu

    _bu.upload_artifacts = lambda tmpdir: str(tmpdir)


def build():
    nc = bacc.Bacc(None, target_bir_lowering=False)

    hT_d = nc.dram_tensor("hT", [D, L], BF, kind="ExternalInput")
    hTm_d = nc.dram_tensor("hTm", [D, TCH], BF, kind="ExternalInput")
    wqT_d = nc.dram_tensor("wqT", [D, JC], BF, kind="ExternalInput")
    wkT_d = nc.dram_tensor("wkT", [D, JC], BF, kind="ExternalInput")
    wvT_d = nc.dram_tensor("wvT", [D, JC], BF, kind="ExternalInput")
    wmT_d = nc.dram_tensor("wmT", [D, HD], BF, kind="ExternalInput")
    woT_d = nc.dram_tensor("woT", [D, D], BF, kind="ExternalInput")
    cst_d = nc.dram_tensor("cst", [1, 1472], BF, kind="ExternalInput")
    st_d = nc.dram_tensor("st", [128, NKT], F32, kind="ExternalInput")
    id_d = nc.dram_tensor("ident", [128, 128], BF, kind="ExternalInput")

    out_rows = nc.dram_tensor("out_rows", [TCH, D], F32, kind="ExternalOutput")
    metric_rows = nc.dram_tensor("metric_rows", [HD, TCH], F32, kind="ExternalOutput")

    with tile.TileContext(nc) as tc:
        with (
            tc.tile_pool(name="sb", bufs=1) as sb,
            tc.tile_pool(name="work", bufs=2) as work,
            tc.tile_pool(name="ps", bufs=1, space="PSUM") as ps,
            tc.tile_pool(name="dram", bufs=1, space="DRAM") as dpool,
        ):
            # ---------------- constants / weights to SBUF ----------------
            cst_sb = sb.tile([1, 1472], BF, name="cst_sb", tag="cst_sb")
            nc.sync.dma_start(out=cst_sb[:], in_=cst_d[:])
            s_sb = sb.tile([128, NKT], F32, name="s_sb", tag="s_sb")
            nc.sync.dma_start(out=s_sb[:], in_=st_d[:])
            ident_sb = sb.tile([128, 128], BF, name="ident_sb", tag="ident_sb")
            nc.sync.dma_start(out=ident_sb[:], in_=id_d[:])

            # tiny warmup AllToAll: the first collective on a NEFF pays a
            # ~20us setup cost; absorb it during the DMA ramp
            wu_in = dpool.tile([NC, 64], BF, name="wu_in", tag="wu_in")
            wu_out = dpool.tile([NC, 64], BF, name="wu_out", tag="wu_out")
            nc.sync.dma_start(out=wu_in[:], in_=cst_d[:, 0:512].rearrange("o (r x) -> (o r) x", r=NC))
            nc.gpsimd.collective_compute(
                "AllToAll",
                mybir.AluOpType.bypass,
                replica_groups=[list(range(NC))],
                ins=[wu_in.opt()],
                outs=[wu_out.opt()],
            )

            ones_sb = sb.tile([1, 512], BF, name="ones_sb", tag="ones_sb")
            nc.vector.memset(ones_sb[:], 1.0)

            wq_sb = sb.tile([128, NDT, JC], BF, name="wq_sb", tag="wq_sb")
            nc.sync.dma_start(out=wq_sb[:], in_=wqT_d[:].rearrange("(a p) j -> p a j", a=NDT))
            wk_sb = sb.tile([128, NDT, JC], BF, name="wk_sb", tag="wk_sb")
            nc.sync.dma_start(out=wk_sb[:], in_=wkT_d[:].rearrange("(a p) j -> p a j", a=NDT))
            wv_sb = sb.tile([128, NDT, JC], BF, name="wv_sb", tag="wv_sb")
            nc.sync.dma_start(out=wv_sb[:], in_=wvT_d[:].rearrange("(a p) j -> p a j", a=NDT))

            hT_sb = []
            for di in range(NDT):
                t = sb.tile([128, L], BF, name=f"hT_sb{di}", tag=f"hT_sb{di}")
                nc.sync.dma_start(out=t[:], in_=hT_d[di * 128:(di + 1) * 128, :])
                hT_sb.append(t)

            wm_sb = sb.tile([128, NDT, HD], BF, name="wm_sb", tag="wm_sb")
            nc.sync.dma_start(out=wm_sb[:], in_=wmT_d[:].rearrange("(a p) j -> p a j", a=NDT))
            hTm_sb = sb.tile([128, NDT, TCH], BF, name="hTm_sb", tag="hTm_sb")
            nc.sync.dma_start(out=hTm_sb[:], in_=hTm_d[:].rearrange("(a p) t -> p a t", a=NDT))
            wo_sb = []
            for c in range(NC):
                t = sb.tile([128, D], BF, name=f"wo_sb{c}", tag=f"wo_sb{c}")
                nc.sync.dma_start(out=t[:], in_=woT_d[c * 128:(c + 1) * 128, :])
                wo_sb.append(t)

            # ------- Q^T, K^T, V^T projections (dims on partitions) -------
            QT_sb = sb.tile([128, L], BF, name="QT_sb", tag="QT_sb")
            KT_sb = sb.tile([128, L], BF, name="KT_sb", tag="KT_sb")
            VT_sb = sb.tile([128, L], BF, name="VT_sb", tag="VT_sb")

            def emit_proj_pass(qch, w_sb, boff, dst):
                cs = [4 * qch + j for j in range(4)]
                pt = []
                for j, c in enumerate(cs):
                    tag = "pj" if j < 2 else "u"
                    pt.append(ps.tile(
                        [128, 512], F32, tag=tag, name=f"p{qch}_{boff}_{j}", bufs=2
                    ))
                for di in range(NDT):
                    for j in range(4):
                        nc.tensor.matmul(
                            pt[j][:], lhsT=w_sb[:, di, :],
                            rhs=hT_sb[di][:, cs[j] * 512:(cs[j] + 1) * 512],
                            start=(di == 0), stop=False,
                        )
                for j in range(4):
                    nc.tensor.matmul(
                        pt[j][:], lhsT=cst_sb[:, boff:boff + 128],
                        rhs=ones_sb[:, 0:512], start=False, stop=True,
                    )
                for j in range(4):
                    nc.vector.tensor_copy(
                        out=dst[:, cs[j] * 512:(cs[j] + 1) * 512], in_=pt[j][:]
                    )

            # ---------------- metric = hTm^T @ wm + bm --------------------
            pm = ps.tile([HD, TCH], F32, tag="pj", name="pm", bufs=2)
            for di in range(NDT):
                nc.tensor.matmul(
                    pm[:], lhsT=wm_sb[:, di, :], rhs=hTm_sb[:, di, :],
                    start=(di == 0), stop=False,
                )
            nc.tensor.matmul(
                pm[:], lhsT=cst_sb[:, CST_BM:CST_BM + HD],
                rhs=ones_sb[:, 0:TCH], start=False, stop=True,
            )
            msb = work.tile([HD, TCH], F32, tag="msb", name="msb")
            nc.vector.tensor_copy(out=msb[:], in_=pm[:])
            nc.sync.dma_start(out=metric_rows[:], in_=msb[:])

            # ------- V natural via TensorE transpose, + aug columns -------
            # Vaug[:, tt, :] = [V0*s (0:64) | s (64) | V1*s (65:129) | s (129)]
            Vaug_sb = sb.tile([128, NKT, 130], BF, name="Vaug_sb", tag="Vaug_sb")

            def emit_vaug(tt):
                pv = ps.tile([128, JC], BF, tag="pj", name=f"pv{tt}", bufs=2)
                nc.tensor.transpose(
                    pv[:], VT_sb[:, tt * 128:(tt + 1) * 128], ident_sb[:]
                )
                sc = s_sb[:, tt:tt + 1]
                nc.vector.tensor_scalar(
                    Vaug_sb[:, tt, 0:64], pv[:, 0:64], sc, None, mybir.AluOpType.mult
                )
                nc.vector.tensor_scalar(
                    Vaug_sb[:, tt, 65:129], pv[:, 64:128], sc, None, mybir.AluOpType.mult
                )
                nc.vector.tensor_copy(out=Vaug_sb[:, tt, 64:65], in_=sc)
                nc.vector.tensor_copy(out=Vaug_sb[:, tt, 129:130], in_=sc)

            # ---------------- attention ----------------------------------
            attnT_sb = sb.tile([128, L], BF, name="attnT_sb", tag="attnT_sb")

            def emit_kt(qc, kt, u0, u1):
                stp = ps.tile([128, 1024], F32, tag="st", name=f"stp{qc}_{kt}", bufs=2)
                nc.tensor.matmul(
                    stp[:, 0:512],
                    lhsT=KT_sb[0:64, kt * 128:(kt + 1) * 128],
                    rhs=QT_sb[0:64, qc * 512:(qc + 1) * 512],
                    start=True, stop=True,
                )
                nc.tensor.matmul(
                    stp[:, 512:1024],
                    lhsT=KT_sb[64:128, kt * 128:(kt + 1) * 128],
                    rhs=QT_sb[64:128, qc * 512:(qc + 1) * 512],
                    start=True, stop=True,
                )
                sts = work.tile([128, 1024], BF, tag="sts", name=f"sts{qc}_{kt}", bufs=3)
                nc.scalar.activation(
                    out=sts[:], in_=stp[:], func=mybir.ActivationFunctionType.Exp
                )
                nc.tensor.matmul(
                    u0[:], lhsT=Vaug_sb[:, kt, 0:65], rhs=sts[:, 0:512],
                    start=(kt == 0), stop=(kt == NKT - 1),
                )
                nc.tensor.matmul(
                    u1[:], lhsT=Vaug_sb[:, kt, 65:130], rhs=sts[:, 512:1024],
                    start=(kt == 0), stop=(kt == NKT - 1),
                )

            def emit_u_copy(qc, u0, u1):
                # copy U out of PSUM right away so the u slots recycle fast
                stash = []
                for h, u in enumerate((u0, u1)):
                    usb = work.tile(
                        [65, 512], F32, tag="usb", name=f"usb{qc}_{h}", bufs=4
                    )
                    nc.vector.tensor_copy(out=usb[:], in_=u[:])
                    stash.append(usb)
                return stash

            def emit_norm(qc, stash):
                # attnT = U / denom; denom row 64; PE not involved
                for h, usb in enumerate(stash):
                    rec = work.tile([1, 512], F32, tag="rec", name=f"rec{qc}_{h}")
                    nc.vector.reciprocal(out=rec[:], in_=usb[64:65, :])
                    bcs = work.tile([64, 512], F32, tag="bcs", name=f"bcs{qc}_{h}")
                    nc.gpsimd.partition_broadcast(bcs[:], rec[:])
                    nc.vector.tensor_tensor(
                        attnT_sb[64 * h:64 * h + 64, qc * 512:(qc + 1) * 512],
                        usb[0:64, :], bcs[:], mybir.AluOpType.mult,
                    )

            # ---- pipelined A2A + out_proj: 4 groups of 2 q-chunks ----
            # group g covers token cols [1024g, 1024g+1024); rank r gets the
            # 128-col slice at 1024g + 128r. core r's out_rows row-block g
            # holds global tokens [1024g + 128r, 1024g + 128r + 128).
            a2a_ins, a2a_outs, L_gs = [], [], []
            for g in range(4):
                t_in = dpool.tile([NC, 128, 128], BF, name=f"a2a_in{g}", tag=f"a2a_in{g}")
                t_out = dpool.tile([NC, 128, 128], BF, name=f"a2a_out{g}", tag=f"a2a_out{g}")
                a2a_ins.append(t_in)
                a2a_outs.append(t_out)

            def emit_a2a(g):
                base = 1024 * g
                nc.sync.dma_start(
                    out=a2a_ins[g][:].rearrange("r p q -> p r q"),
                    in_=attnT_sb[:, base:base + 1024].rearrange(
                        "p (r q) -> p r q", r=NC
                    ),
                )
                nc.gpsimd.collective_compute(
                    "AllToAll",
                    mybir.AluOpType.bypass,
                    replica_groups=[list(range(NC))],
                    ins=[a2a_ins[g].opt()],
                    outs=[a2a_outs[g].opt()],
                )
                L_g = sb.tile([128, NC, 128], BF, name=f"L_sb{g}", tag=f"L_sb{g}")
                nc.sync.dma_start(
                    out=L_g[:], in_=a2a_outs[g][:].rearrange("r p q -> p r q")
                )
                L_gs.append(L_g)

            def emit_outproj(g):
                L_g = L_gs[g]
                po = [
                    ps.tile([128, 512], F32, tag="pj", name=f"po{g}_{ec}", bufs=2)
                    for ec in range(D // 512)
                ]
                for c in range(NC):
                    for ec in range(D // 512):
                        nc.tensor.matmul(
                            po[ec][:], lhsT=L_g[:, c, :],
                            rhs=wo_sb[c][:, ec * 512:(ec + 1) * 512],
                            start=(c == 0), stop=False,
                        )
                for ec in range(D // 512):
                    nc.tensor.matmul(
                        po[ec][:], lhsT=ones_sb[:, 0:128],
                        rhs=cst_sb[:, CST_BO + ec * 512:CST_BO + (ec + 1) * 512],
                        start=False, stop=True,
                    )
                for ec in range(D // 512):
                    osb = work.tile([128, 512], F32, tag="osb", name=f"osb{g}_{ec}")
                    nc.vector.tensor_copy(out=osb[:], in_=po[ec][:])
                    nc.sync.dma_start(
                        out=out_rows[g * 128:(g + 1) * 128, ec * 512:(ec + 1) * 512],
                        in_=osb[:],
                    )

            for qch in range(NQC // 4):
                for w_sb, boff, dst in (
                    (wq_sb, CST_BQ, QT_sb),
                    (wk_sb, CST_BK, KT_sb),
                    (wv_sb, CST_BV, VT_sb),
                ):
                    emit_proj_pass(qch, w_sb, boff, dst)
                for tt in range(16 * qch, 16 * qch + 16):
                    emit_vaug(tt)
            prev = None
            for qc in range(NQC):
                u0 = ps.tile([65, 512], F32, tag="u", name=f"u0_{qc}", bufs=2)
                u1 = ps.tile([65, 512], F32, tag="u", name=f"u1_{qc}", bufs=2)
                for kt in range(NKT):
                    emit_kt(qc, kt, u0, u1)
                    if kt == 6 and prev is not None:
                        emit_norm(qc - 1, prev)
                        prev = None
                        if qc % 2 == 0 and qc >= 2:
                            emit_a2a((qc - 1) // 2)
                    if kt == 26 and qc % 2 == 1 and qc >= 3:
                        emit_outproj((qc - 3) // 2)
                prev = emit_u_copy(qc, u0, u1)
            emit_norm(NQC - 1, prev)
            emit_a2a(3)
            emit_outproj(3)

    if not nc.is_finalized():
        nc.finalize()
    return nc


def _prep_inputs(hidden_states, size, wq, bq, wk, bk, wv, bv, wo, bo):
    h = np.asarray(hidden_states, dtype=np.float32)[0]          # (L, D)
    s = np.asarray(size, dtype=np.float32)[0, 0]                # (L,)
    wq = np.asarray(wq, dtype=np.float32)
    wk = np.asarray(wk, dtype=np.float32)
    wv = np.asarray(wv, dtype=np.float32)
    wo = np.asarray(wo, dtype=np.float32)
    bq = np.asarray(bq, dtype=np.float32)
    bk = np.asarray(bk, dtype=np.float32)
    bv = np.asarray(bv, dtype=np.float32)
    bo = np.asarray(bo, dtype=np.float32)

    hT = np.ascontiguousarray(h.T).astype(BF16)                 # (D, L)
    woT = np.ascontiguousarray(wo.T).astype(BF16)               # (D, D)
    wm = wk.reshape(H, HD, D).mean(axis=0)                      # (HD, D)
    bm = bk.reshape(H, HD).mean(axis=0)                         # (HD,)
    wmT = np.ascontiguousarray(wm.T).astype(BF16)               # (D, HD)
    s_t = np.ascontiguousarray(s.reshape(NKT, 128).T).astype(np.float32)
    ident = np.eye(128, dtype=np.float32).astype(BF16)

    in_maps = []
    for c in range(NC):
        rows = slice(c * JC, (c + 1) * JC)
        wqc = wq[rows] * SCALE
        cst = np.concatenate(
            [bq[rows] * SCALE, bk[rows], bv[rows], bm, bo]
        ).reshape(1, -1).astype(BF16)
        in_maps.append({
            "hT": hT,
            "hTm": np.ascontiguousarray(hT[:, c * TCH:(c + 1) * TCH]),
            "wqT": np.ascontiguousarray(wqc.T).astype(BF16),
            "wkT": np.ascontiguousarray(wk[rows].T).astype(BF16),
            "wvT": np.ascontiguousarray(wv[rows].T).astype(BF16),
            "wmT": wmT,
            "woT": woT,
            "cst": cst,
            "st": s_t,
            "ident": ident,
        })
    return in_maps


def kernel(hidden_states, size, wq, bq, wk, bk, wv, bv, wo, bo):
    global LAST_EXEC_NS, LAST_RESULTS
    if "nc" not in _CACHED:
        _CACHED["nc"] = build()
    nc = _CACHED["nc"]

    in_maps = _prep_inputs(hidden_states, size, wq, bq, wk, bk, wv, bv, wo, bo)

    trace = bool(int(os.environ.get("AURORA_TRACE", "0")))
    if trace:
        _install_ntff_shim()
    res = run_bass_kernel_spmd(
        nc, in_maps, core_ids=list(range(NC)), trace=trace
    )
    LAST_RESULTS = res
    LAST_EXEC_NS = res.exec_time_ns

    outs = res.results
    out = np.empty((1, L, D), dtype=np.float32)
    for r in range(NC):
        rows = np.asarray(outs[r]["out_rows"], dtype=np.float32)
        for g in range(4):
            base = 1024 * g + 128 * r
            out[0, base:base + 128, :] = rows[g * 128:(g + 1) * 128, :]
    met = np.concatenate(
        [np.asarray(outs[c]["metric_rows"], dtype=np.float32) for c in range(NC)],
        axis=1,
    ).T[None]                                                   # (1, L, HD)
    return out, met
